# revision 47
# baseline (speedup 1.0000x reference)
"""Complex multi-head attention on 8 Trainium2 cores (Bass/Tile), v3.

Sharding: pure data-parallel over batch (B=8 -> 1 batch per core),
weights replicated. No collectives.

Engine-balance design (vs the 341.5us baseline):
  - ACT paces attention; its per-op overhead is halved by PAIR-Exps:
    each (tk) score pair (comp r + comp i) lands in one 2-bank PSUM tile
    [128,1024], one Exp serves both comps (8 Exps per group, not 16).
  - Softmax denominators: e-pair tiles are pair-summed (4 adds per comp
    per group, split DVE/Pool), then 4 ones-matmuls per comp reduce the
    partials in a dedicated 1-bank sums pool DURING THE NEXT GROUP
    (k-slotted, so nothing stalls); rec = Exp(-Ln(sums)) on ACT;
    normalization runs two groups later at k=2.
  - Q/K/V projections use Karatsuba (3 half-size mults); combines are
    4 DVE ops per subblock (s1 evac -> u, w2 -> v) compatible with a
    single rotating PSUM bank; per-head stacks distributed via
    SBUF->SBUF DMA half-copies.
  - kswap trick: score matmuls use K-side variants (kneg=[kr;-ki],
    kswap=[ki;kr]) against a single qstack.
  - Next-pair Q/K projection matmuls interleave one per attention
    iteration (generator), emitted BEFORE the e-waiting AV matmuls so the
    PE never drains while ACT produces each exp pair; pair 0 interleaves
    into the V-projection phase.
  - v1 is a single 4D tile; v2 ([-vi|vr]) per head is 2 strided DVE ops.
  - bf16 operands on the PE everywhere; p1c/p2c/norm/rec fp32.
  - O projection: schoolbook over pair-stacked (otr/oti) outputs with
    (o,c)-interleaved weight columns -> PSUM == [S, D, 2] DRAM layout.
  - PSUM budget: st-pairs 2x2 + p12 2 + prj 1 + sums 1 = 8 banks.
  - Input DMA descriptors round-robin across 3 engine queues.
"""

import sys
import types
import numpy as np

B, S, D, H = 8, 1024, 512, 8
DH = D // H
NCORES = 8
NPAIR = 4  # head pairs

LAST_EXEC_NS = None


# ---------------------------------------------------------------- shims
def _install_axon_profile_shim():
    if "antenv.axon_hooks" in sys.modules:
        return
    try:
        import antenv  # noqa: F401

        mod = types.ModuleType("antenv.axon_hooks")
        state = {"hook": None}
        mod.set_axon_ntff_profile_hook = lambda h: state.__setitem__("hook", h)
        mod.get_axon_ntff_profile_hook = lambda: state["hook"]
        sys.modules["antenv.axon_hooks"] = mod
        from trn_agent_boot.trn_boot import _ntff_profile_via_ctypes

        hook = _ntff_profile_via_ctypes("/opt/axon/libaxon_pjrt.so")
        if hook is not None:
            mod.set_axon_ntff_profile_hook(hook)
    except Exception:
        pass


def _install_tile_drain_patch():
    """This walrus build allows ONE sync wait per instruction; split the
    TileContext exit drain's waits across preceding sync NOPs."""
    import concourse.mybir as mybir
    import concourse.tile as tile
    from concourse.vector_clock import ScopedClock

    if getattr(tile.TileContext, "_drain_patched", False):
        return

    def _patched(self, tick_clock, wait_clock):
        probe = mybir.InstNoOp(name="I-drain-probe")
        probe.engine = mybir.EngineType.SP
        wait_clock.add_sem_waits(probe, ScopedClock({None: tick_clock.global_clock}))
        waits = list(probe.sync_info.on_wait or []) if probe.sync_info else []
        for w in waits:
            nop = self.nc.sync.nop()
            nop.ins.sync_info = mybir.SyncInfo(on_wait=[w], on_update=[])
        self.nc.sync.drain()
        self.nc.all_engine_barrier()
        assert self.sems is not None
        popped = self.nc._tile_sem_poison_stack.pop()
        assert popped is self._sem_poison
        self.nc.clear_and_free_semaphores(list(self.sems.allocated().values()))
        self.nc.all_engine_barrier()

    tile.TileContext._drain_and_barrier = _patched
    tile.TileContext._drain_patched = True


def _split_waits(nc, max_waits=1):
    """Hoist extra sync waits onto preceding same-engine NOPs (walrus here
    rejects >1 sync wait per instruction)."""
    import concourse.mybir as mybir

    def process(blk):
        lst = blk.instructions
        i = 0
        while i < len(lst):
            inst = lst[i]
            if hasattr(inst, "blocks"):
                for b in inst.blocks or []:
                    process(b)
            si = inst.sync_info
            if si is not None and si.on_wait and len(si.on_wait) > max_waits:
                waits = list(si.on_wait)
                keep, extra = waits[-max_waits:], waits[:-max_waits]
                inst.sync_info = mybir.SyncInfo(
                    on_wait=keep, on_update=list(si.on_update or [])
                )
                for j, w in enumerate(extra):
                    nop = mybir.InstNoOp(name=f"{inst.name}-ws{j}")
                    nop.engine = inst.engine
                    nop.sync_info = mybir.SyncInfo(on_wait=[w], on_update=[])
                    lst.insert(i, nop)
                    i += 1
            i += 1

    for f in nc.m.functions:
        for blk in f.blocks:
            process(blk)


# ------------------------------------------------------------ host prep
def _qk_w(wr, wi, s):
    """Karatsuba Q/K weights: [4 pairs, 128, 12*128], cols (tj, kk).
    lhsT layout: [k=in-feat chunk 128, m=pair out-feats 128]."""
    W1 = wr.T * s
    W2 = wi.T * s
    out = np.empty((NPAIR, 128, 1024), np.float32)
    for p in range(NPAIR):
        csl = slice(p * 128, (p + 1) * 128)
        for tj, W in enumerate((W1, W2)):
            blk = W[:, csl]  # [512, 128]
            for kk in range(4):
                c0 = (tj * 4 + kk) * 128
                out[p][:, c0 : c0 + 128] = blk[kk * 128 : (kk + 1) * 128]
    return out


def _v_w(wvr, wvi):
    """Karatsuba V weights (rhs): [2, 128, 4*512], cols (kk, n).
    The wr+wi plane is summed on-device."""
    out = np.empty((2, 128, 2048), np.float32)
    for tj, W in enumerate((wvr.T, wvi.T)):
        for kk in range(4):
            out[tj][:, kk * 512 : (kk + 1) * 512] = W[kk * 128 : (kk + 1) * 128, :]
    return out


def _o_w(wor, woi):
    """O-proj schoolbook over pair stacks: [4 pairs, 2 (A,B), 128, 1024].
    A rows = or-features, B rows = oi-features; cols (o,c) interleaved."""
    out = np.empty((NPAIR, 2, 128, 1024), np.float32)
    for p in range(NPAIR):
        dsl = slice(p * 128, (p + 1) * 128)
        out[p, 0, :, 0::2] = wor[:, dsl].T
        out[p, 0, :, 1::2] = woi[:, dsl].T
        out[p, 1, :, 0::2] = -woi[:, dsl].T
        out[p, 1, :, 1::2] = wor[:, dsl].T
    return out


def _x12(x):
    """[S, D, 2] -> [8, 128, S] feature-major: xr chunks 0-3, xi 4-7.
    The Karatsuba xs = xr+xi chunks are computed on-device (Pool) to cut
    3MB off the bandwidth-bound input preamble."""
    xr = x[:, :, 0].T
    xi = x[:, :, 1].T
    out = np.empty((8, 128, S), np.float32)
    out[0:4] = xr.reshape(4, 128, S)
    out[4:8] = xi.reshape(4, 128, S)
    return out


# ------------------------------------------------------------ bass build
def _build_nc():
    import concourse.bass as bass
    import concourse.mybir as mybir
    import concourse.tile as tile
    from contextlib import ExitStack

    F32 = mybir.dt.float32
    BF16 = mybir.dt.bfloat16
    EXP = mybir.ActivationFunctionType.Exp
    LN = mybir.ActivationFunctionType.Ln

    nc = bass.Bass()
    d_xq = nc.dram_tensor("xq", [8, 128, S], BF16, kind="ExternalInput")
    d_xk = nc.dram_tensor("xk", [8, 128, S], BF16, kind="ExternalInput")
    d_xv = nc.dram_tensor("xv", [8, 128, S], BF16, kind="ExternalInput")
    d_wq = nc.dram_tensor("wq", [NPAIR, 128, 1024], BF16, kind="ExternalInput")
    d_wk = nc.dram_tensor("wk", [NPAIR, 128, 1024], BF16, kind="ExternalInput")
    d_wv = nc.dram_tensor("wv", [2, 128, 2048], BF16, kind="ExternalInput")
    d_wo = nc.dram_tensor("wo", [NPAIR, 2, 128, 1024], BF16, kind="ExternalInput")
    d_cst = nc.dram_tensor("cst", [128, 128], BF16, kind="ExternalInput")
    d_out = nc.dram_tensor("out", [S, 1024], F32, kind="ExternalOutput")

    with tile.TileContext(nc) as tc, ExitStack() as ctx:
        ctx.enter_context(
            nc.allow_low_precision(reason="bf16 operands validated vs 2e-2 gate")
        )
        pXQ = ctx.enter_context(tc.tile_pool(name="xq", bufs=12))
        pXK = ctx.enter_context(tc.tile_pool(name="xk", bufs=12))
        pBig = ctx.enter_context(tc.tile_pool(name="big", bufs=12))  # xtv -> otr/oti
        pV1 = ctx.enter_context(tc.tile_pool(name="v1", bufs=1))
        pV2 = ctx.enter_context(tc.tile_pool(name="v2", bufs=2))
        pStk = ctx.enter_context(tc.tile_pool(name="stk", bufs=12))
        pWqk = ctx.enter_context(tc.tile_pool(name="wqk", bufs=4))
        pWv = ctx.enter_context(tc.tile_pool(name="wv", bufs=3))
        pE = ctx.enter_context(tc.tile_pool(name="e", bufs=4))
        pAcc = ctx.enter_context(tc.tile_pool(name="acc", bufs=10))
        pPC = ctx.enter_context(tc.tile_pool(name="pc", bufs=5))
        pRec = ctx.enter_context(tc.tile_pool(name="rec", bufs=3))
        pOt = ctx.enter_context(tc.tile_pool(name="ot", bufs=3))
        pTmpB = ctx.enter_context(tc.tile_pool(name="tmpb", bufs=4))
        pTmpF = ctx.enter_context(tc.tile_pool(name="tmpf", bufs=3))
        pOev = ctx.enter_context(tc.tile_pool(name="oev", bufs=2))

        ps_st = ctx.enter_context(tc.tile_pool(name="ps_st", bufs=2, space="PSUM"))
        ps_p12 = ctx.enter_context(tc.tile_pool(name="ps_p12", bufs=2, space="PSUM"))
        ps_prj = ctx.enter_context(tc.tile_pool(name="ps_prj", bufs=1, space="PSUM"))
        ps_sums = ctx.enter_context(
            tc.tile_pool(name="ps_sums", bufs=1, space="PSUM")
        )

        # ---- input DMA, round-robin across engine queues, need-ordered ----
        issuers = [nc.sync, nc.scalar, nc.gpsimd]
        dma_i = [0]

        def dma(out, in_):
            issuers[dma_i[0] % 3].dma_start(out=out, in_=in_)
            dma_i[0] += 1

        pC = ctx.enter_context(tc.tile_pool(name="cst", bufs=1))
        ones = pC.tile([128, 128], BF16, tag="cst", name="ones")
        wqk_t = {}

        def dma_wqk(p):
            if p >= NPAIR:
                return
            tq = pWqk.tile([128, 1536], BF16, tag="wqk")
            dma(tq[:, 0:1024], d_wq[p])
            tk_ = pWqk.tile([128, 1536], BF16, tag="wqk")
            dma(tk_[:, 0:1024], d_wk[p])
            for t in (tq, tk_):
                nc.gpsimd.tensor_add(
                    t[:, 1024:1536], t[:, 0:512], t[:, 512:1024]
                )
            wqk_t[p] = (tq, tk_)

        # strict need-order: the preamble is HBM-bandwidth-bound (~11MB),
        # so each tensor is issued just before its first consumer slot:
        # wv0+xtv(xr) gate the first V matmul, wqk0+xtq gate gen0's Q
        # side (interleaved into V), xtk gates gen0's K side (~t_=4).
        dma(ones, d_cst[:, :])
        wv_t = [pWv.tile([128, 2048], BF16, tag="wv", name=f"wv{j}") for j in range(3)]
        xtv = [
            pBig.tile([128, S], BF16, tag="big", name=f"xtv{c}") for c in range(12)
        ]
        dma(wv_t[0], d_wv[0])
        for c in range(4):
            dma(xtv[c], d_xv[c])
        dma(wv_t[1], d_wv[1])
        for c in range(4, 8):
            dma(xtv[c], d_xv[c])
        nc.gpsimd.tensor_add(wv_t[2], wv_t[0], wv_t[1])
        for c in range(4):
            nc.gpsimd.tensor_add(xtv[8 + c], xtv[c], xtv[4 + c])
        dma_wqk(0)
        xtq = [pXQ.tile([128, S], BF16, tag="xq", name=f"xq{c}") for c in range(12)]
        xtk = [pXK.tile([128, S], BF16, tag="xk", name=f"xk{c}") for c in range(12)]
        for c in range(8):
            dma(xtq[c], d_xq[c])
            dma(xtk[c], d_xk[c])
        for c in range(4):
            nc.gpsimd.tensor_add(xtq[8 + c], xtq[c], xtq[4 + c])
            nc.gpsimd.tensor_add(xtk[8 + c], xtk[c], xtk[4 + c])

        # ---- per-head Q/K stacks via Karatsuba generator ----
        qstack, kneg, kswap = {}, {}, {}

        def qk_gen(p, pools=None):
            """Yields once per tensor matmul; combines/DMA emitted inline.
            Rotating PSUM bank(s): t1 evacuated (s1) before t2 starts;
            u/w2 consume t2 before t3 starts. With 2 pools (V phase, where
            ps_sums is idle) consecutive products never share a bank and
            the chain never waits on the DVE combines."""
            pl = pools or [(ps_prj, "ps_prj")]
            pcnt = [0]

            def ptile():
                pool, tg = pl[pcnt[0] % len(pl)]
                t = pool.tile([128, 512], F32, tag=tg, name="prj")
                pcnt[0] += 1
                return t

            h0, h1 = 2 * p, 2 * p + 1
            for h in (h0, h1):
                qstack[h] = pStk.tile([128, S], BF16, tag="stk", name=f"qs{h}")
                kneg[h] = pStk.tile([128, S], BF16, tag="stk", name=f"kn{h}")
                kswap[h] = pStk.tile([128, S], BF16, tag="stk", name=f"kw{h}")
            for side in range(2):
                wt = wqk_t[p][side]
                xt = xtq if side == 0 else xtk
                for nh in range(2):
                    nsl = slice(nh * 512, (nh + 1) * 512)
                    t1 = ptile()
                    for kk in range(4):
                        nc.tensor.matmul(
                            t1,
                            lhsT=wt[:, kk * 128 : (kk + 1) * 128],
                            rhs=xt[kk][:, nsl],
                            start=(kk == 0),
                            stop=(kk == 3),
                        )
                        yield
                    s1 = pTmpF.tile([128, 512], F32, tag="tmpf")
                    nc.vector.tensor_copy(s1, t1)
                    t2 = ptile()
                    for kk in range(4):
                        nc.tensor.matmul(
                            t2,
                            lhsT=wt[:, (4 + kk) * 128 : (5 + kk) * 128],
                            rhs=xt[4 + kk][:, nsl],
                            start=(kk == 0),
                            stop=(kk == 3),
                        )
                        if kk == 3:
                            u = pTmpB.tile([128, 512], BF16, tag="tmpb")
                            nc.vector.tensor_sub(u, s1, t2)
                            w2 = pTmpF.tile([128, 512], F32, tag="tmpf")
                            nc.vector.tensor_add(w2, s1, t2)
                        yield
                    t3 = ptile()
                    for kk in range(4):
                        nc.tensor.matmul(
                            t3,
                            lhsT=wt[:, (8 + kk) * 128 : (9 + kk) * 128],
                            rhs=xt[8 + kk][:, nsl],
                            start=(kk == 0),
                            stop=(kk == 3),
                        )
                        yield
                    v = pTmpB.tile([128, 512], BF16, tag="tmpb")
                    nc.vector.tensor_sub(v, t3, w2)
                    # distribute halves to per-head stacks (SBUF->SBUF DMA)
                    if side == 0:
                        for i, h in enumerate((h0, h1)):
                            hs = slice(i * 64, (i + 1) * 64)
                            nc.sync.dma_start(out=qstack[h][0:64, nsl], in_=u[hs, :])
                            nc.sync.dma_start(out=qstack[h][64:128, nsl], in_=v[hs, :])
                    else:
                        vneg = pTmpB.tile([128, 512], BF16, tag="tmpb")
                        nc.vector.tensor_scalar_mul(vneg, v, -1.0)
                        for i, h in enumerate((h0, h1)):
                            hs = slice(i * 64, (i + 1) * 64)
                            nc.sync.dma_start(out=kneg[h][0:64, nsl], in_=u[hs, :])
                            nc.sync.dma_start(
                                out=kneg[h][64:128, nsl], in_=vneg[hs, :]
                            )
                            nc.sync.dma_start(out=kswap[h][0:64, nsl], in_=v[hs, :])
                            nc.sync.dma_start(out=kswap[h][64:128, nsl], in_=u[hs, :])

        # ---- V projection (Karatsuba), all heads ----
        # v1 = [128 tok-in-chunk, 8 t_, 8 heads, (vr 64 | vi 64)] bf16
        gen0 = [None]

        def gen0_pump(n):
            if gen0[0] is None:
                return
            for _ in range(n):
                if next(gen0[0], "END") == "END":
                    gen0[0] = None
                    return

        v1big = pV1.tile([128, 8, 8, 128], BF16, tag="v1", name="v1big")
        gen0[0] = qk_gen(0, [(ps_prj, "ps_prj"), (ps_sums, "ps_sums")])
        for t_ in range(8):
            tsl = slice(t_ * 128, (t_ + 1) * 128)
            v1t = v1big[:, t_]
            t1 = ps_p12.tile([128, 512], F32, tag="ps_p12")
            for kk in range(4):
                nc.tensor.matmul(
                    t1,
                    lhsT=xtv[kk][:, tsl],
                    rhs=wv_t[0][:, kk * 512 : (kk + 1) * 512],
                    start=(kk == 0),
                    stop=(kk == 3),
                )
            # evacuate t1 (frees its bank for t3; 2-buf p12 pool; also
            # avoids illegal 2-PSUM-input tensor ops in the combines)
            s1 = pTmpF.tile([128, 512], F32, tag="tmpf")
            nc.vector.tensor_copy(s1, t1)
            gen0_pump(2)
            t2 = ps_p12.tile([128, 512], F32, tag="ps_p12")
            for kk in range(4):
                nc.tensor.matmul(
                    t2,
                    lhsT=xtv[4 + kk][:, tsl],
                    rhs=wv_t[1][:, kk * 512 : (kk + 1) * 512],
                    start=(kk == 0),
                    stop=(kk == 3),
                )
            # vr = t1 - t2 = s1 - t2; vi = t3 - (t1 + t2) = t3 - w2
            nc.vector.tensor_sub(v1t[:, :, 0:64], s1, t2)
            gen0_pump(2)
            w2 = pTmpF.tile([128, 512], F32, tag="tmpf")
            nc.vector.tensor_add(w2, s1, t2)
            t3 = ps_st.tile([128, 512], F32, tag="ps_st", name="vt3")
            for kk in range(4):
                nc.tensor.matmul(
                    t3,
                    lhsT=xtv[8 + kk][:, tsl],
                    rhs=wv_t[2][:, kk * 512 : (kk + 1) * 512],
                    start=(kk == 0),
                    stop=(kk == 3),
                )
            nc.vector.tensor_sub(v1t[:, :, 64:128], t3, w2)
            gen0_pump(2)

        # drain whatever of pair 0 the V phase didn't cover
        if gen0[0] is not None:
            for _ in gen0[0]:
                pass
            gen0[0] = None
        dma_wqk(1)

        # v2h: [-vi | vr] per head, [128, 8 tk, 128]; 2 strided Pool ops
        v2h = {}

        def emit_v2h(h):
            if h >= H:
                return
            vt = pV2.tile([128, 8, 128], BF16, tag="v2", name=f"v2h{h}")
            nc.vector.tensor_scalar_mul(vt[:, :, 0:64], v1big[:, :, h, 64:128], -1.0)
            nc.vector.tensor_copy(vt[:, :, 64:128], v1big[:, :, h, 0:64])
            v2h[h] = vt

        emit_v2h(0)

        # otr/oti pair stacks (attention output, O-proj input)
        otr = [
            pBig.tile([128, S], BF16, tag="big", name=f"otr{i}") for i in range(NPAIR)
        ]
        oti = [
            pBig.tile([128, S], BF16, tag="big", name=f"oti{i}") for i in range(NPAIR)
        ]

        # Deferred pipeline queue: group g's pair-partials are reduced by
        # 4 ones-matmuls per comp into the 1-bank sums pool during group
        # g+1 (k slots 0-3 / 8-11), Ln at k=5/13 and rec=Exp(-Ln) at
        # k=7/15 on ACT, and the normalization at group g+2's k=2.
        gq = []

        def emit_sums_step(ent, k):
            if k in (0, 1, 2, 3):
                if k == 0:
                    ent["sums_r"] = ps_sums.tile(
                        [128, 512], F32, tag="ps_sums", name="sums_r"
                    )
                nc.tensor.matmul(
                    ent["sums_r"],
                    lhsT=ones,
                    rhs=ent["pr"][k],
                    start=(k == 0),
                    stop=(k == 3),
                )
            elif k == 5:
                lnt = pTmpF.tile([128, 512], F32, tag="tmpf", name="lnr")
                nc.scalar.activation(lnt, ent["sums_r"], func=LN)
                ent["lnr"] = lnt
            elif k == 7:
                rc = pRec.tile([128, 512], F32, tag="rec")
                nc.scalar.activation(rc, ent["lnr"], func=EXP, scale=-1.0)
                ent["rr"] = rc
            elif k in (8, 9, 10, 11):
                if k == 8:
                    ent["sums_i"] = ps_sums.tile(
                        [128, 512], F32, tag="ps_sums", name="sums_i"
                    )
                nc.tensor.matmul(
                    ent["sums_i"],
                    lhsT=ones,
                    rhs=ent["pi"][k - 8],
                    start=(k == 8),
                    stop=(k == 11),
                )
            elif k == 13:
                lnt = pTmpF.tile([128, 512], F32, tag="tmpf", name="lni")
                nc.scalar.activation(lnt, ent["sums_i"], func=LN)
                ent["lni"] = lnt
            elif k == 15:
                rc = pRec.tile([128, 512], F32, tag="rec")
                nc.scalar.activation(rc, ent["lni"], func=EXP, scale=-1.0)
                ent["ri"] = rc

        def emit_norm(ent):
            otf = pOt.tile([128, 512], BF16, tag="ot")
            tn = pOt.tile([128, 512], F32, tag="ot")
            nc.vector.tensor_mul(otf, ent["p1c"], ent["rr"])
            nc.vector.tensor_mul(tn, ent["p2c"], ent["ri"])
            nc.vector.tensor_add(otf, otf, tn)
            hs = slice(ent["half"] * 64, (ent["half"] + 1) * 64)
            nc.sync.dma_start(out=otr[ent["p"]][hs, ent["qsl"]], in_=otf[0:64, :])
            nc.sync.dma_start(out=oti[ent["p"]][hs, ent["qsl"]], in_=otf[64:128, :])

        # ---- attention, head-major, pair-Exp groups ----
        gen = [None]
        for h in range(H):
            p = h // 2
            if h % 2 == 0 and p + 1 < NPAIR:
                gen[0] = qk_gen(p + 1)
                dma_wqk(p + 2)
            if h == 6:
                wo_t = []
                for pp in range(NPAIR):
                    for side in range(2):
                        t = pXQ.tile([128, 1024], BF16, tag="xq", name="wo")
                        nc.sync.dma_start(out=t, in_=d_wo[pp, side])
                        wo_t.append(t)
            for nh in range(2):
                qsl = slice(nh * 512, (nh + 1) * 512)
                p1 = ps_p12.tile([128, 512], F32, tag="ps_p12")
                p2 = ps_p12.tile([128, 512], F32, tag="ps_p12")
                stp = [None, None]

                def emit_st_pair(j):
                    st = ps_st.tile([128, 1024], F32, tag="ps_st", name="stp")
                    ksl = slice(j * 128, (j + 1) * 128)
                    nc.tensor.matmul(
                        st[:, 0:512],
                        lhsT=kneg[h][:, ksl],
                        rhs=qstack[h][:, qsl],
                        start=True,
                        stop=True,
                    )
                    nc.tensor.matmul(
                        st[:, 512:1024],
                        lhsT=kswap[h][:, ksl],
                        rhs=qstack[h][:, qsl],
                        start=True,
                        stop=True,
                    )
                    stp[j % 2] = st

                ent = {"p": p, "half": h % 2, "qsl": qsl, "pr": [], "pi": []}
                epairs = []

                def hook(k):
                    if gen[0] is not None and (
                        k % 4 != 3 or (h % 2 == 1 and nh == 1)
                    ):
                        if next(gen[0], "END") == "END":
                            gen[0] = None
                    if nh == 0 and k == 5:
                        emit_v2h(h + 1)
                    if k == 2 and gq and gq[0].get("ri") is not None:
                        emit_norm(gq.pop(0))
                    if gq:
                        emit_sums_step(gq[-1], k)

                emit_st_pair(0)
                for j in range(8):
                    if j + 1 < 8:
                        emit_st_pair(j + 1)
                    ep = pE.tile([128, 1024], BF16, tag="e", name="ep")
                    nc.scalar.activation(ep, stp[j % 2], func=EXP)
                    epairs.append(ep)
                    # independent matmuls (gen/sums) queue BEFORE the AVs so
                    # the PE stays busy while ACT produces this e-pair
                    hook(2 * j)
                    hook(2 * j + 1)
                    nc.tensor.matmul(
                        p1,
                        lhsT=v1big[:, j, h, :],
                        rhs=ep[:, 0:512],
                        start=(j == 0),
                        stop=(j == 7),
                    )
                    nc.tensor.matmul(
                        p2,
                        lhsT=v2h[h][:, j, :],
                        rhs=ep[:, 512:1024],
                        start=(j == 0),
                        stop=(j == 7),
                    )
                    if j % 2 == 1:
                        # pair partials; DVE for j 1/5, Pool for j 3/7
                        eng = nc.vector if j in (1, 5) else nc.gpsimd
                        ar = pAcc.tile([128, 512], BF16, tag="acc", name="ar")
                        eng.tensor_add(
                            ar, epairs[j - 1][:, 0:512], epairs[j][:, 0:512]
                        )
                        ent["pr"].append(ar)
                        ai = pAcc.tile([128, 512], BF16, tag="acc", name="ai")
                        eng.tensor_add(
                            ai, epairs[j - 1][:, 512:1024], epairs[j][:, 512:1024]
                        )
                        ent["pi"].append(ai)
                    if j == 7:
                        # free the p banks ASAP for the next group
                        ent["p1c"] = pPC.tile([128, 512], F32, tag="pc", name="p1c")
                        nc.vector.tensor_copy(ent["p1c"], p1)
                        ent["p2c"] = pPC.tile([128, 512], F32, tag="pc", name="p2c")
                        nc.vector.tensor_copy(ent["p2c"], p2)
                gq.append(ent)
            if h % 2 == 1 and gen[0] is not None:
                for _ in gen[0]:
                    pass
                gen[0] = None
        # flush: second-to-last norm, then last group's sums/recs/norm
        emit_norm(gq.pop(0))
        for k in (0, 1, 2, 3, 5, 7, 8, 9, 10, 11, 13, 15):
            emit_sums_step(gq[0], k)
        emit_norm(gq.pop(0))

        # ---- output projection (schoolbook over pair stacks) ----
        opools = [ps_p12, ps_prj, ps_p12, ps_sums]
        oi = [0]
        for t_ in range(8):
            tsl = slice(t_ * 128, (t_ + 1) * 128)
            for nhf in range(2):
                nsl = slice(nhf * 512, (nhf + 1) * 512)
                pool = opools[oi[0] % 4]
                oi[0] += 1
                ps = pool.tile(
                    [128, 512], F32,
                    tag=["ps_p12", "ps_prj", "ps_p12", "ps_sums"][(oi[0] - 1) % 4],
                    name="ops",
                )
                for pp in range(NPAIR):
                    nc.tensor.matmul(
                        ps,
                        lhsT=otr[pp][:, tsl],
                        rhs=wo_t[2 * pp][:, nsl],
                        start=(pp == 0),
                        stop=False,
                    )
                    nc.tensor.matmul(
                        ps,
                        lhsT=oti[pp][:, tsl],
                        rhs=wo_t[2 * pp + 1][:, nsl],
                        start=False,
                        stop=(pp == 3),
                    )
                oev = pOev.tile([128, 512], F32, tag="oev")
                nc.scalar.copy(oev, ps)
                nc.sync.dma_start(out=d_out[tsl, nsl], in_=oev)

    _split_waits(nc)
    return nc


_NC_CACHE = {}


def kernel(
    queries,
    keys,
    values,
    wq_r,
    wq_i,
    wk_r,
    wk_i,
    wv_r,
    wv_i,
    wo_r,
    wo_i,
    _trace=False,
):
    global LAST_EXEC_NS
    _install_axon_profile_shim()
    _install_tile_drain_patch()
    from concourse.bass_utils import run_bass_kernel_spmd

    import ml_dtypes

    bf16 = ml_dtypes.bfloat16
    scale = 1.0 / np.sqrt(DH)
    WQ = _qk_w(np.asarray(wq_r), np.asarray(wq_i), scale).astype(bf16)
    WK = _qk_w(np.asarray(wk_r), np.asarray(wk_i), 1.0).astype(bf16)
    WV = _v_w(np.asarray(wv_r), np.asarray(wv_i)).astype(bf16)
    WO = _o_w(np.asarray(wo_r), np.asarray(wo_i)).astype(bf16)
    CST = np.ones((128, 128), bf16)

    queries = np.asarray(queries)
    keys = np.asarray(keys)
    values = np.asarray(values)

    in_maps = []
    for b in range(NCORES):
        in_maps.append(
            {
                "xq": _x12(queries[b]).astype(bf16),
                "xk": _x12(keys[b]).astype(bf16),
                "xv": _x12(values[b]).astype(bf16),
                "wq": WQ,
                "wk": WK,
                "wv": WV,
                "wo": WO,
                "cst": CST,
            }
        )

    if "nc" not in _NC_CACHE:
        _NC_CACHE["nc"] = _build_nc()
    nc = _NC_CACHE["nc"]

    res = run_bass_kernel_spmd(nc, in_maps, list(range(NCORES)), trace=_trace)
    LAST_EXEC_NS = res.exec_time_ns

    out = np.empty((B, S, D, 2), np.float32)
    for b in range(NCORES):
        out[b] = res.results[b]["out"].reshape(S, D, 2)
    return out
